# revision 3
# baseline (speedup 1.0000x reference)
"""TRN2 Bass kernel for nn_Attention_41506563948971.

Reference computation (per batch b):
    G  = (q @ w + b) @ a^T          [Lq, La]
    P  = softmax(G, axis=q)         (softmax over dim=1, the q axis)
    out= P^T @ q                    [La, H]

Sharding: data-parallel over batch B=8 across the 8 NeuronCores; w
replicated. Each core computes one full batch; no collectives. The bias b
is constant along the softmax (q) axis, so it cancels exactly in P and
never reaches the output — it is dropped entirely.

Numerics: logits G have sigma ~= 1024 (q,a ~ N(0,1), H=1024); the dim-q
softmax is near-one-hot (top-2 gap ~ Exponential(mean ~250)), so logit
errors flip argmax columns with probability ~eps/250 each. The 2e-2
rel-err budget allows logit noise eps ~ 0.03. MM1/MM2 therefore run as
split matmuls: an exact fp16 hi*hi pass at 1 cycle/row, plus the two
fp16-residual cross terms in fp8 at 0.5 cycles/row (MatmulPerfMode
.DoubleRow packs 2 k-chunks per instruction), i.e. 2.0 effective
passes instead of the 3 fp16 passes a pure-fp16 hi/lo scheme needs.
Balanced power-of-2 operand scalings keep every fp8 product at its true
scale, so all passes accumulate into one PSUM bank:
  MM1 (E5M2, dynamic range needs 5 exp bits):  (w*2^-7)(q_lo*2^7) and
      (w_lo*2^7)(q*2^-7); per-pass rounding ~7.2% of a ~0.004-sigma
      cross => logit noise ~0.018.
  MM2 (E4M3): (a_lo*2^9)(Qw*2^-9) and (a*2^-3)(Qw_lo*2^3); ~6.5%
      rounding of ~0.13-sigma crosses => ~0.017.
Total eps ~ 0.025 => expected <1 argmax flip over all 16K columns,
rel err ~1e-2 worst case vs the 2e-2 gate. MM3's operands are one-hot-ish
softmax weights and q in fp16 (~2e-4 output error). The softmax 1/sum is
folded into a per-partition scale on MM3's small output.

Schedule notes:
- PE cycles/core: MM1 196K + MM2 393K + MM3 262K ~= 852K ~= 355us at
  2.4GHz; every other engine is kept under that.
- DMA queues: a dma_start blocks its issuing sequencer for the whole
  transfer, so issue is spread out: bulk loads/stores on Pool (SWDGE,
  ~25ns seq cost), q/a hi+lo fp16 xbar transposes on SP, the per-iter
  softmax E^T transpose (single full-width call) on ACT.
- fp8 operands are produced by DVE/ACT scaled copies from the already-
  transposed fp16 tiles (the xbar cannot transpose 1-byte dtypes).
- ~28 warmup matmuls pre-ramp the HAM clock gate to 2.4 GHz.
- MM2 runs nq-outer so each GT chunk's reduce_max overlaps the next
  chunk's matmuls; exps are emitted ahead of MM3's scales on ACT's
  in-order queue; MM3 is software-pipelined one a-tile behind.
"""

import sys

sys.path.insert(0, "/opt/trn_rl_repo")

from contextlib import ExitStack

import numpy as np

import concourse.bass as bass
import concourse.bacc as bacc
import concourse.mybir as mybir
import concourse.tile as tile
from concourse.masks import make_identity

dt = mybir.dt
AF = mybir.ActivationFunctionType
OP = mybir.AluOpType
AX = mybir.AxisListType
DR = mybir.MatmulPerfMode.DoubleRow

P = 128
H = 1024
KO = H // P          # 8 contraction chunks
LQ = 2048
LA = 2048
NQT = LQ // P        # 16 q row-tiles
NAT = LA // P        # 16 a row-tiles
QC = 512             # free-dim chunk (one fp32 PSUM bank)
NQC = LQ // QC       # 4
B = 8                # batch == number of cores

F16 = dt.float16
E4 = dt.float8e4     # E4M3
E5 = dt.float8e5     # E5M2

S1 = 2.0 ** 7        # MM1 cross scaling (E5M2): lo*S1 paired with full*1/S1
S2A = 2.0 ** 9       # MM2 cross1: a_lo*S2A with Qw*1/S2A
S2B = 2.0 ** 3       # MM2 cross2: Qw_lo*S2B with a*1/S2B


def _trace_kernel(tc, q_d, a_d, w_d, o_d):
    nc = tc.nc
    with ExitStack() as ctx:
        pp = ctx.enter_context(tc.tile_pool(name="persist", bufs=1))
        # "scratch" serves the PE-transpose banks (phase-1 startup), the
        # warmup, and MM3's output banks — lifetimes never overlap.
        ps_pool = ctx.enter_context(tc.tile_pool(name="ps", bufs=6, space="PSUM"))
        scratch = ctx.enter_context(tc.tile_pool(name="scratch", bufs=2, space="PSUM"))
        tp_pool = scratch
        op_pool = scratch

        id_sp = pp.tile([P, P], F16, tag="id_sp")
        make_identity(nc, id_sp[:])

        # PE clock warmup: the HAM gate holds the PE at 1.2 GHz until it
        # sees ~3.4 us of sustained activity.
        warm_sb = pp.tile([P, P], F16, tag="warm_sb")
        nc.vector.memset(warm_sb[:], 1.0)
        warm_ps = op_pool.tile([P, P], dt.float32, tag="tp", name="warm_ps")
        NWARM = 28
        for j in range(NWARM):
            nc.tensor.matmul(
                warm_ps[:], warm_sb[:], warm_sb[:],
                start=(j == 0), stop=(j == NWARM - 1),
            )

        # QwT = (q @ w)^T in [h, q] layout: fp16 hi + fp8 E4M3 hi/lo.
        qwt_hi = pp.tile([P, KO, LQ], F16, tag="qwt_hi")
        qw8_hi = pp.tile([P, KO, LQ], E4, tag="qw8_hi")   # Qw * 2^-9
        qw8_lo = pp.tile([P, KO, LQ], E4, tag="qw8_lo")   # (Qw - hi16) * 2^3
        # q in natural [q, h] layout, fp16 (hi split; also MM3's rhs).
        q_r = pp.tile([P, NQT, H], F16, tag="q_r")

        # ---------------- Phase 1: MM1 -> QwT ----------------
        with ExitStack() as p1:
            wpool = p1.enter_context(tc.tile_pool(name="wpool", bufs=1))
            stage = p1.enter_context(tc.tile_pool(name="stage", bufs=3))
            split = p1.enter_context(tc.tile_pool(name="split", bufs=2))
            mlo = p1.enter_context(tc.tile_pool(name="mlo", bufs=2))
            qtp = p1.enter_context(tc.tile_pool(name="qtp", bufs=2))
            ltp = p1.enter_context(tc.tile_pool(name="ltp", bufs=3))

            w_hi = wpool.tile([P, KO, H], F16, tag="w_hi")
            w8_hi = wpool.tile([P, KO, H], E5, tag="w8_hi")   # w * 2^-7
            w8_lo = wpool.tile([P, KO, H], E5, tag="w8_lo")   # w_lo * 2^7

            def load_w(k):
                wt = stage.tile([P, H], dt.float32, tag="wstage", name=f"wt{k}")
                nc.gpsimd.dma_start(wt[:], w_d[k * P:(k + 1) * P, :])
                nc.vector.tensor_copy(w_hi[:, k], wt[:])
                nc.scalar.activation(w8_hi[:, k], wt[:], AF.Identity,
                                     scale=1.0 / S1)
                wlo = split.tile([P, H], F16, tag="wlo", name=f"wlo{k}")
                nc.vector.tensor_tensor(wlo[:], wt[:], w_hi[:, k], OP.subtract)
                nc.vector.tensor_scalar_mul(w8_lo[:, k], wlo[:], S1)

            def alloc_qt(qc):
                qt_hi = qtp.tile([P, KO, QC], F16, tag="qt_hi", name=f"qth{qc}")
                q8_hi = qtp.tile([P, KO, QC], E5, tag="q8_hi", name=f"q8h{qc}")
                q8_lo = qtp.tile([P, KO, QC], E5, tag="q8_lo", name=f"q8l{qc}")
                return qt_hi, q8_hi, q8_lo

            def prep_q_tile(qc, t, qt, use_pe=False):
                qt_hi, q8_hi, q8_lo = qt
                qs = stage.tile([P, H], dt.float32, tag="qstage",
                                name=f"qs{qc}_{t}")
                row0 = qc * QC + t * P
                nc.gpsimd.dma_start(qs[:], q_d[row0:row0 + P, :])
                idx = qc * (QC // P) + t
                # hi split written straight into q_r (it IS fp16(q))
                nc.vector.tensor_copy(q_r[:, idx], qs[:])
                qlo = split.tile([P, H], F16, tag="qlo", name=f"ql{qc}_{t}")
                nc.vector.tensor_tensor(qlo[:], qs[:], q_r[:, idx], OP.subtract)
                ts_ = slice(t * P, (t + 1) * P)
                lt = ltp.tile([P, KO, P], F16, tag="lt", name=f"lt{qc}_{t}")
                if use_pe:
                    # PE transposes, batched 8 per PSUM bank with one
                    # strided DVE evacuation
                    for src, dst in ((q_r[:, idx], qt_hi[:, :, ts_]),
                                     (qlo[:], lt[:])):
                        tp = tp_pool.tile([P, KO * P], F16, tag="tp")
                        for k in range(KO):
                            nc.tensor.transpose(
                                tp[:, k * P:(k + 1) * P],
                                src[:, k * P:(k + 1) * P],
                                id_sp[:],
                            )
                        nc.vector.tensor_copy(
                            dst, tp[:].rearrange("p (k c) -> p k c", k=KO)
                        )
                else:
                    # xbar DMA transpose: out[p, k, j] = in[j, k*128+p]
                    nc.sync.dma_start_transpose(qt_hi[:, :, ts_], q_r[:, idx])
                    nc.sync.dma_start_transpose(lt[:], qlo[:])
                # fp8 operands from the transposed fp16 tiles
                nc.scalar.activation(q8_hi[:, :, ts_], qt_hi[:, :, ts_],
                                     AF.Identity, scale=1.0 / S1)
                nc.vector.tensor_scalar_mul(q8_lo[:, :, ts_], lt[:], S1)

            # q-chunk 0's loads/splits/transposes first so PE starts
            # immediately; w loads overlap the transposes.
            qt_cur = alloc_qt(0)
            for t in range(QC // P):
                prep_q_tile(0, t, qt_cur, use_pe=True)
            for k in range(KO):
                load_w(k)

            for qc in range(NQC):
                qt_hi, q8_hi, q8_lo = qt_cur
                if qc + 1 < NQC:
                    qt_next = alloc_qt(qc + 1)
                for m in range(KO):
                    ms = slice(m * P, (m + 1) * P)
                    acc = ps_pool.tile([P, QC], dt.float32, tag="ps")
                    for k in range(KO):
                        nc.tensor.matmul(
                            acc[:], w_hi[:, k, ms], qt_hi[:, k, :],
                            start=(k == 0), stop=False,
                        )
                    for kk in range(KO // 2):
                        kp = slice(2 * kk, 2 * kk + 2)
                        nc.tensor.matmul(
                            acc[:], w8_hi[:, kp, ms], q8_lo[:, kp, :],
                            start=False, stop=False, perf_mode=DR,
                        )
                    for kk in range(KO // 2):
                        kp = slice(2 * kk, 2 * kk + 2)
                        nc.tensor.matmul(
                            acc[:], w8_lo[:, kp, ms], q8_hi[:, kp, :],
                            start=False, stop=(kk == KO // 2 - 1), perf_mode=DR,
                        )
                    # Qw operand set: hi16+hi8 on ACT, lo16+lo8 on DVE
                    cs = slice(qc * QC, (qc + 1) * QC)
                    dhi = qwt_hi[:, m, cs]
                    nc.scalar.copy(dhi, acc[:])
                    nc.scalar.activation(qw8_hi[:, m, cs], acc[:], AF.Identity,
                                         scale=1.0 / S2A)
                    lo = mlo.tile([P, QC], F16, tag="mlo", name=f"lo{qc}_{m}")
                    nc.vector.tensor_tensor(lo[:], acc[:], dhi, OP.subtract)
                    nc.vector.tensor_scalar_mul(qw8_lo[:, m, cs], lo[:], S2B)
                    # interleave the next chunk's per-tile prep between
                    # m-blocks so DMA/DVE work lands just ahead of use
                    if qc + 1 < NQC and m < QC // P:
                        prep_q_tile(qc + 1, m, qt_next, use_pe=(qc == 0))
                if qc + 1 < NQC:
                    qt_cur = qt_next

        # ---------------- Phase 2: MM2 + softmax + MM3 ----------------
        with ExitStack() as p2:
            astage = p2.enter_context(tc.tile_pool(name="astage", bufs=4))
            asplit = p2.enter_context(tc.tile_pool(name="asplit", bufs=3))
            atp = p2.enter_context(tc.tile_pool(name="atp", bufs=2))
            altp = p2.enter_context(tc.tile_pool(name="altp", bufs=2))
            ppool = p2.enter_context(tc.tile_pool(name="ppool", bufs=2))
            ptpool = p2.enter_context(tc.tile_pool(name="ptpool", bufs=2))
            outp = p2.enter_context(tc.tile_pool(name="outp", bufs=2))
            redp = p2.enter_context(tc.tile_pool(name="redp", bufs=4))

            def prep_a_tile(i, use_pe=False):
                at = astage.tile([P, H], dt.float32, tag="astage", name=f"at{i}")
                nc.gpsimd.dma_start(at[:], a_d[i * P:(i + 1) * P, :])
                a_hi = asplit.tile([P, H], F16, tag="a_hi", name=f"ah{i}")
                a_lo = asplit.tile([P, H], F16, tag="a_lo", name=f"al{i}")
                nc.vector.tensor_copy(a_hi[:], at[:])
                nc.vector.tensor_tensor(a_lo[:], at[:], a_hi[:], OP.subtract)
                at_hi = atp.tile([P, KO, P], F16, tag="at_hi", name=f"ath{i}")
                a8_hi = atp.tile([P, KO, P], E4, tag="a8_hi", name=f"a8h{i}")
                a8_lo = atp.tile([P, KO, P], E4, tag="a8_lo", name=f"a8l{i}")
                alt = altp.tile([P, KO, P], F16, tag="alt", name=f"alt{i}")
                if use_pe:
                    for src, dst in ((a_hi, at_hi), (a_lo, alt)):
                        tp = tp_pool.tile([P, KO * P], F16, tag="tp")
                        for k in range(KO):
                            nc.tensor.transpose(
                                tp[:, k * P:(k + 1) * P],
                                src[:, k * P:(k + 1) * P],
                                id_sp[:],
                            )
                        nc.vector.tensor_copy(
                            dst[:], tp[:].rearrange("p (k c) -> p k c", k=KO)
                        )
                else:
                    nc.sync.dma_start_transpose(at_hi[:], a_hi[:])
                    nc.sync.dma_start_transpose(alt[:], a_lo[:])
                nc.scalar.activation(a8_hi[:], at_hi[:], AF.Identity,
                                     scale=1.0 / S2B)
                nc.vector.tensor_scalar_mul(a8_lo[:], alt[:], S2A)
                return at_hi, a8_hi, a8_lo

            def do_mm3(pt_sb, rinv, i):
                # MM3: out[a, h] = sum_q ET[q, a] * q[q, h], then * (1/sum)
                o_sb = outp.tile([P, H], dt.float32, tag="o_sb", name=f"osb{i}")
                for nh in range(H // QC):
                    acc = op_pool.tile([P, QC], dt.float32, tag="tp")
                    for t in range(NQT):
                        nc.tensor.matmul(
                            acc[:],
                            pt_sb[:, t, :],
                            q_r[:, t, nh * QC:(nh + 1) * QC],
                            start=(t == 0),
                            stop=(t == NQT - 1),
                        )
                    # 1/sum scale on ScalarE (Identity supports AP scale)
                    nc.scalar.activation(
                        o_sb[:, nh * QC:(nh + 1) * QC], acc[:], AF.Identity,
                        scale=rinv[:],
                    )
                nc.gpsimd.dma_start(o_d[i * P:(i + 1) * P, :], o_sb[:])

            at_cur = prep_a_tile(0, use_pe=True)
            mm3_prev = None

            for i in range(NAT):
                at_hi, a8_hi, a8_lo = at_cur

                # MM2 nq-outer: each GT chunk finishes early so its
                # reduce_max overlaps the next chunk's matmuls.
                gt = []
                gmax = redp.tile([P, NQC], dt.float32, tag="gmax")
                for nq in range(NQC):
                    cs = slice(nq * QC, (nq + 1) * QC)
                    g = ps_pool.tile([P, QC], dt.float32, tag="ps",
                                     name=f"gt{nq}")
                    for k in range(KO):
                        nc.tensor.matmul(
                            g[:], at_hi[:, k, :], qwt_hi[:, k, cs],
                            start=(k == 0), stop=False,
                        )
                    for kk in range(KO // 2):
                        kp = slice(2 * kk, 2 * kk + 2)
                        nc.tensor.matmul(
                            g[:], a8_lo[:, kp, :], qw8_hi[:, kp, cs],
                            start=False, stop=False, perf_mode=DR,
                        )
                    for kk in range(KO // 2):
                        kp = slice(2 * kk, 2 * kk + 2)
                        nc.tensor.matmul(
                            g[:], a8_hi[:, kp, :], qw8_lo[:, kp, cs],
                            start=False, stop=(kk == KO // 2 - 1), perf_mode=DR,
                        )
                    nc.vector.reduce_max(gmax[:, nq:nq + 1], g[:], axis=AX.X)
                    gt.append(g)

                negm = redp.tile([P, 1], dt.float32, tag="negm")
                nc.vector.reduce_max(negm[:], gmax[:], axis=AX.X, negate=True)

                # exps first so they're ahead of MM3's scales on ACT's
                # in-order queue
                p_sb = ppool.tile([P, LQ], dt.float16, tag="p_sb")
                sums = redp.tile([P, NQC], dt.float32, tag="sums")
                for nq in range(NQC):
                    nc.scalar.activation(
                        p_sb[:, nq * QC:(nq + 1) * QC],
                        gt[nq][:],
                        AF.Exp,
                        bias=negm[:],
                        scale=1.0,
                        accum_out=sums[:, nq:nq + 1],
                    )
                sall = redp.tile([P, 1], dt.float32, tag="sall")
                nc.vector.reduce_sum(sall[:], sums[:], axis=AX.X)
                rinv = redp.tile([P, 1], dt.float32, tag="rinv")
                nc.vector.reciprocal(rinv[:], sall[:])

                # PE work that needs no softmax results fills the window
                # while ACT runs the exps: next a-tile's transposes, then
                # the previous iteration's MM3.
                if i + 1 < NAT:
                    at_next = prep_a_tile(i + 1)
                if mm3_prev is not None:
                    do_mm3(*mm3_prev)

                # transpose E=[a,q] -> ET=[q,a] via xbar DMA (one call)
                pt_sb = ptpool.tile([P, NQT, P], dt.float16, tag="pt_sb")
                nc.scalar.dma_start_transpose(pt_sb[:], p_sb[:])

                mm3_prev = (pt_sb, rinv, i)
                if i + 1 < NAT:
                    at_cur = at_next

            do_mm3(*mm3_prev)


_CACHE = {}


def build_nc():
    if "nc" in _CACHE:
        return _CACHE["nc"]
    nc = bacc.Bacc("TRN2", target_bir_lowering=False, debug=False)
    q_d = nc.dram_tensor("q", [LQ, H], dt.float32, kind="ExternalInput").ap()
    a_d = nc.dram_tensor("a", [LA, H], dt.float32, kind="ExternalInput").ap()
    w_d = nc.dram_tensor("w", [H, H], dt.float32, kind="ExternalInput").ap()
    o_d = nc.dram_tensor("o", [LA, H], dt.float32, kind="ExternalOutput").ap()
    with tile.TileContext(nc) as tc:
        _trace_kernel(tc, q_d, a_d, w_d, o_d)
    nc.compile()
    _CACHE["nc"] = nc
    return nc


def get_runner():
    """Build (once) a cached jitted SPMD executable over the 8 cores.

    Mirrors bass2jax.run_bass_via_pjrt's multi-core path, but caches the
    jitted callable so repeated invocations don't recompile.
    """
    if "runner" in _CACHE:
        return _CACHE["runner"]
    import jax
    from jax.sharding import Mesh, PartitionSpec
    from jax.experimental.shard_map import shard_map

    from concourse import bass2jax

    nc = build_nc()
    bass2jax.install_neuronx_cc_hook()

    partition_name = nc.partition_id_tensor.name if nc.partition_id_tensor else None
    in_names, out_names, out_avals, zero_outs = [], [], [], []
    for alloc in nc.m.functions[0].allocations:
        if not isinstance(alloc, mybir.MemoryLocationSet):
            continue
        name = alloc.memorylocations[0].name
        if alloc.kind == "ExternalInput":
            if name != partition_name:
                in_names.append(name)
        elif alloc.kind == "ExternalOutput":
            shape = tuple(alloc.tensor_shape)
            dtype = mybir.dt.np(alloc.dtype)
            out_names.append(name)
            out_avals.append(jax.core.ShapedArray(shape, dtype))
            zero_outs.append(np.zeros(shape, dtype))
    n_params = len(in_names)
    all_in_names = list(in_names) + list(out_names)
    if partition_name is not None:
        all_in_names.append(partition_name)

    def _body(*args):
        operands = list(args)
        if partition_name is not None:
            operands.append(bass2jax.partition_id_tensor())
        outs = bass2jax._bass_exec_p.bind(
            *operands,
            out_avals=tuple(out_avals),
            in_names=tuple(all_in_names),
            out_names=tuple(out_names),
            lowering_input_output_aliases=(),
            sim_require_finite=True,
            sim_require_nnan=True,
            nc=nc,
        )
        return tuple(outs)

    devices = jax.devices()[:B]
    mesh = Mesh(np.asarray(devices), ("core",))
    n_outs = len(out_names)
    in_specs = (PartitionSpec("core"),) * (n_params + n_outs)
    out_specs = (PartitionSpec("core"),) * n_outs
    sharded = jax.jit(
        shard_map(
            _body, mesh=mesh, in_specs=in_specs, out_specs=out_specs, check_rep=False
        ),
        keep_unused=True,
    )
    runner = (sharded, in_names, out_names, out_avals, zero_outs)
    _CACHE["runner"] = runner
    return runner


def run_cores(in_maps):
    """Run the kernel SPMD over 8 cores; in_maps is a list of 8 dicts."""
    sharded, in_names, out_names, out_avals, zero_outs = get_runner()
    concat_in = [
        np.concatenate([np.asarray(m[name]) for m in in_maps], axis=0)
        for name in in_names
    ]
    concat_zeros = [
        np.zeros((B * z.shape[0], *z.shape[1:]), z.dtype) for z in zero_outs
    ]
    out_arrs = sharded(*concat_in, *concat_zeros)
    return [
        {
            name: np.asarray(out_arrs[j]).reshape(B, *out_avals[j].shape)[c]
            for j, name in enumerate(out_names)
        }
        for c in range(B)
    ]


def kernel(q, a, w, b):
    q = np.ascontiguousarray(np.asarray(q, dtype=np.float32))
    a = np.ascontiguousarray(np.asarray(a, dtype=np.float32))
    w = np.ascontiguousarray(np.asarray(w, dtype=np.float32))
    assert q.shape == (B, LQ, H) and a.shape == (B, LA, H)
    assert w.shape == (H, H)

    in_maps = [{"q": q[i], "a": a[i], "w": w} for i in range(B)]
    try:
        from concourse.bass_utils import run_bass_kernel_spmd

        results = run_bass_kernel_spmd(
            build_nc(), in_maps, core_ids=list(range(B))
        ).results
    except Exception:
        # fallback: cached jitted shard_map runner (same execution path)
        results = run_cores(in_maps)
    return np.stack([results[i]["o"] for i in range(B)], axis=0)
